# revision 13
# baseline (speedup 1.0000x reference)
"""Trainium2 Bass kernel for nn_CLFormer (3-block linear-attention transformer).

Sharding: pure data parallel — batch 32 split as 4 per NeuronCore across 8
cores; all parameters replicated; outputs concatenated.

Per-core layout: 4 batches x 32 channels packed on the 128 SBUF partitions
("channel-major" [128=4bx32c, L]). The kv-gram contracts over tokens, so a
token-major copy is produced per block (block 0: PE transposes of f32 x;
blocks 1-2: DRAM round-trip DMA transpose of the bf16 gelu output). The
token-major value operand carries a ones column (129-wide chunks) so the
gram matmul also produces the k-softmax denominator for free. Attention-out
and FC1 fuse into M1 = diag(1/ksum) @ G @ W1; FC matmuls run as single
128-wide matmuls with block-diagonal weights.
"""
import sys
import numpy as np

for _p in ("/opt/trn_rl_repo", "/root/.axon_site/_ro/trn_rl_repo"):
    if _p not in sys.path:
        sys.path.append(_p)

from contextlib import ExitStack

import concourse.bass as bass
import concourse.mybir as mybir
import bass_rust
from concourse import tile
from concourse.masks import make_identity
from concourse.bass_utils import run_bass_kernel_spmd

F32 = mybir.dt.float32
BF16 = mybir.dt.bfloat16
AF = mybir.ActivationFunctionType
MUL = mybir.AluOpType.mult
ADD = mybir.AluOpType.add

P = 128
B_LOC = 4            # batches per core
C = 32               # channels
L = 16384            # sequence length
NB = 3               # transformer blocks
DOUT = 10
HEADS = 4
DH = 8
BN_EPS = 1e-5

SLC = 2048           # slice width (tokens per pipeline slice)
NSL = L // SLC       # 8 slices
NCH = SLC // 128     # 16 chunks per slice
EXT = 129            # chunk width in the ones-extended token-major h tile
ZW = 1024            # phase-2 tile width (tokens per z/gelu tile)
NZ = L // ZW         # 16 phase-2 steps
DEBUG = False
DBG_BLK = 0


# ---------------------------------------------------------------- waitfix --
_WF_SKIP = {"InstEventSemaphore"}
_wf_ctr = [0]


def _fix_sync_waits(nc):
    """Hoist excess sync waits onto InstEventSemaphore (this walrus build
    accepts only 1 wait per instruction). The event-sem executes on the same
    engine stream immediately before, preserving semantics."""
    for fn in nc.m.functions:
        new_blocks = []
        for blk in fn.blocks:
            out = []
            for ins in blk.instructions:
                tname = type(ins).__name__
                si = ins.sync_info
                if si is None or tname in _WF_SKIP:
                    out.append(ins)
                    continue
                waits = list(si.on_wait)
                if len(waits) <= 1:
                    out.append(ins)
                    continue
                keep = waits[-1:]
                excess = waits[:-1]
                for i in range(0, len(excess), 2):
                    chunk = excess[i:i + 2]
                    _wf_ctr[0] += 1
                    ev = mybir.InstEventSemaphore(
                        name=f"wfix{_wf_ctr[0]}", ins=[], outs=[])
                    ev.engine = ins.engine
                    ev.sync_info = mybir.SyncInfo(on_wait=chunk, on_update=[])
                    out.append(ev)
                ins.sync_info = mybir.SyncInfo(
                    on_wait=keep, on_update=list(si.on_update))
                out.append(ins)
            nb = bass_rust.BasicBlock(name=blk.name, instructions=out)
            new_blocks.append(nb)
        fn.blocks = new_blocks


# ---------------------------------------------------------------- program --
def _load_rep(nc, pool, src_ap, cols, dtype, tag=None):
    """DRAM [32, cols] (or [32]-vector viewed [32,1]) -> SBUF [128, cols],
    replicated across the 4 batch partition strips via one broadcast-source
    DMA."""
    stage = pool.tile([P, cols], F32, tag=f"{tag}_st")
    for b in range(B_LOC):
        nc.sync.dma_start(stage[C * b:C * (b + 1), :], src_ap)
    if dtype == F32:
        return stage
    out = pool.tile([P, cols], dtype, tag=f"{tag}_bf")
    nc.vector.tensor_copy(out[:], stage[:])
    return out


def build_program(reps=1):
    nc = bass.Bass()

    x_d = nc.declare_dram_parameter("x", [B_LOC, C, L], F32, isOutput=False)
    fcW1_d = nc.declare_dram_parameter("fcW1", [NB, C, C], F32, isOutput=False)
    fcb1_d = nc.declare_dram_parameter("fcb1", [NB, C], F32, isOutput=False)
    fcW2_d = nc.declare_dram_parameter("fcW2", [NB, C, C], F32, isOutput=False)
    fcb2_d = nc.declare_dram_parameter("fcb2", [NB, C], F32, isOutput=False)
    Wh_d = nc.declare_dram_parameter("Wh", [C, C], F32, isOutput=False)
    bh_d = nc.declare_dram_parameter("bh", [C], F32, isOutput=False)
    bng_d = nc.declare_dram_parameter("bn_gamma", [C], F32, isOutput=False)
    bnb_d = nc.declare_dram_parameter("bn_beta", [C], F32, isOutput=False)
    bnm_d = nc.declare_dram_parameter("bn_mean", [C], F32, isOutput=False)
    bnv_d = nc.declare_dram_parameter("bn_var", [C], F32, isOutput=False)
    Wf_d = nc.declare_dram_parameter("Wf", [C, DOUT], F32, isOutput=False)
    bf_d = nc.declare_dram_parameter("bf", [DOUT], F32, isOutput=False)
    out_d = nc.declare_dram_parameter("out", [B_LOC, DOUT], F32, isOutput=True)
    if DEBUG:
        dbg_w1 = nc.declare_dram_parameter("dbg_w1", [P, C], BF16, isOutput=True)
        dbg_he = nc.declare_dram_parameter("dbg_he", [P, NCH * EXT], BF16, isOutput=True)
        dbg_et = nc.declare_dram_parameter("dbg_et", [P, SLC], BF16, isOutput=True)
        dbg_G = nc.declare_dram_parameter("dbg_G", [P, EXT], F32, isOutput=True)
        dbg_ksC = nc.declare_dram_parameter("dbg_ksC", [P, 1], F32, isOutput=True)
        dbg_M1 = nc.declare_dram_parameter("dbg_M1", [P, P], BF16, isOutput=True)
        dbg_q = nc.declare_dram_parameter("dbg_q", [P, SLC], BF16, isOutput=True)
        dbg_h1 = nc.declare_dram_parameter("dbg_h1", [P, SLC], BF16, isOutput=True)

    with ExitStack() as ctx:
        tc = ctx.enter_context(tile.TileContext(nc))
        cst = ctx.enter_context(tc.tile_pool(name="cst", bufs=1))
        xst = ctx.enter_context(tc.tile_pool(name="xst", bufs=3))
        hcm = ctx.enter_context(tc.tile_pool(name="hcm", bufs=3))
        hex_ = ctx.enter_context(tc.tile_pool(name="hex", bufs=8))
        etm = ctx.enter_context(tc.tile_pool(name="etm", bufs=3))
        qtm = ctx.enter_context(tc.tile_pool(name="qtm", bufs=2))
        sqp = ctx.enter_context(tc.tile_pool(name="sqp", bufs=2))
        bigq = ctx.enter_context(tc.tile_pool(name="bigq", bufs=2))
        a1p = ctx.enter_context(tc.tile_pool(name="a1p", bufs=3))
        smal = ctx.enter_context(tc.tile_pool(name="smal", bufs=2))
        m1p = ctx.enter_context(tc.tile_pool(name="m1p", bufs=2))
        gps = ctx.enter_context(tc.tile_pool(name="gps", bufs=1, space="PSUM"))
        zps = ctx.enter_context(tc.tile_pool(name="zps", bufs=2, space="PSUM"))
        qps = ctx.enter_context(tc.tile_pool(name="qps", bufs=2, space="PSUM"))
        tps = ctx.enter_context(tc.tile_pool(name="tps", bufs=1, space="PSUM"))

        for _rep in range(reps):
            # ---- x loads first: keep the sync queue clear for the ramp ----
            x_cm = x_d[:].rearrange("b c l -> (b c) l")
            xs_tiles = []
            for s in range(NSL):
                xs = xst.tile([P, SLC], F32, tag="xs")
                nc.sync.dma_start(xs[:], x_cm[:, SLC * s:SLC * (s + 1)])
                xs_tiles.append(xs)

            # ---- constants ---------------------------------------------
            if _rep == 0:
                ident = cst.tile([P, P], BF16)
                make_identity(nc, ident[:])
                ident32 = cst.tile([P, P], F32)
                make_identity(nc, ident32[:])
                headmask = cst.tile([P, P], BF16)
                nc.vector.memset(headmask[:], 1.0)
                hm_v = headmask[:].rearrange("p (g i) -> p g i", i=DH)
                nc.gpsimd.affine_select(
                    out=hm_v, in_=hm_v, pattern=[[-DH, P // DH], [0, DH]],
                    compare_op=mybir.AluOpType.is_ge, fill=0.0,
                    base=0, channel_multiplier=1)
                nc.gpsimd.affine_select(
                    out=hm_v, in_=hm_v, pattern=[[DH, P // DH], [0, DH]],
                    compare_op=mybir.AluOpType.is_ge, fill=0.0,
                    base=DH - 1, channel_multiplier=-1)

                # per-block interleave so block 0's weights land first
                W1rep, W2rep, b1rep, b2rep = [], [], [], []
                for i in range(NB):
                    W1rep.append(
                        _load_rep(nc, cst, fcW1_d[i], C, BF16, tag=f"w1_{i}"))
                    b1rep.append(
                        _load_rep(nc, cst, fcb1_d[i].unsqueeze(-1), 1, F32,
                                  tag=f"b1_{i}"))
                    W2rep.append(
                        _load_rep(nc, cst, fcW2_d[i], C, BF16, tag=f"w2_{i}"))
                    b2rep.append(
                        _load_rep(nc, cst, fcb2_d[i].unsqueeze(-1), 1, F32,
                                  tag=f"b2_{i}"))
                Whrep = _load_rep(nc, cst, Wh_d[:], C, F32, tag="wh")
                Wfrep = _load_rep(nc, cst, Wf_d[:], DOUT, F32, tag="wf")
                bh_r = _load_rep(nc, cst, bh_d[:].unsqueeze(-1), 1, F32, tag="bh")
                bng_r = _load_rep(nc, cst, bng_d[:].unsqueeze(-1), 1, F32, tag="bng")
                bnb_r = _load_rep(nc, cst, bnb_d[:].unsqueeze(-1), 1, F32, tag="bnb")
                bnm_r = _load_rep(nc, cst, bnm_d[:].unsqueeze(-1), 1, F32, tag="bnm")
                bnv_r = _load_rep(nc, cst, bnv_d[:].unsqueeze(-1), 1, F32, tag="bnv")
                if DEBUG:
                    nc.sync.dma_start(dbg_w1[:], W1rep[0][:])
                bf_s = cst.tile([P, 1], F32)
                nc.vector.memset(bf_s[:], 0.0)
                for b in range(B_LOC):
                    nc.sync.dma_start(
                        bf_s[C * b:C * b + DOUT, :], bf_d[:].unsqueeze(-1))

                # block-diagonal W2 (one-time): W2blk[32b+c, 32b+e] = W2[c, e]
                W2blk = []
                for i in range(NB):
                    wb = cst.tile([P, P], BF16, tag=f"w2b_{i}")
                    nc.gpsimd.memset(wb[:], 0.0)
                    for b in range(B_LOC):
                        sl = slice(C * b, C * (b + 1))
                        nc.vector.tensor_copy(wb[sl, C * b:C * (b + 1)],
                                              W2rep[i][sl, :])
                    W2blk.append(wb)

                # BN eval folding: y_bn = y_raw * svecL + tvec, where y_raw is
                # the UNSCALED pooled-sum matmul output (missing bias and 1/L).
                eps_t = cst.tile([P, 1], F32)
                nc.vector.memset(eps_t[:], BN_EPS)
                sq_t = cst.tile([P, 1], F32)
                nc.scalar.activation(sq_t[:], bnv_r[:], AF.Sqrt, bias=eps_t[:])
                rs_t = cst.tile([P, 1], F32)
                nc.vector.reciprocal(rs_t[:], sq_t[:])
                svec = cst.tile([P, 1], F32)
                nc.vector.tensor_tensor(svec[:], rs_t[:], bng_r[:], op=MUL)
                svecL = cst.tile([P, 1], F32)
                nc.vector.tensor_scalar_mul(svecL[:], svec[:], 1.0 / L)
                t0 = cst.tile([P, 1], F32)
                nc.vector.tensor_tensor(t0[:], bh_r[:], bnm_r[:],
                                        op=mybir.AluOpType.subtract)
                t1 = cst.tile([P, 1], F32)
                nc.vector.tensor_tensor(t1[:], t0[:], svec[:], op=MUL)
                tvec = cst.tile([P, 1], F32)
                nc.vector.tensor_tensor(tvec[:], t1[:], bnb_r[:], op=ADD)

            pooled_parts = cst.tile([P, NZ], F32, tag="pool")

            h_dr_tiles = []      # DRAM bf16 h tiles (blocks 1, 2 input)

            for blk in range(NB):
                # ===================== phase 1 (token-major) ============
                G_ps = gps.tile([P, EXT], F32, tag="G")
                q_cm = bigq.tile([P, L], BF16, tag="qcm")
                for s in range(NSL):
                    if blk == 0:
                        # token-major via cast (DVE) + PE transpose of x chunks
                        he = hex_.tile([P, NCH * EXT], BF16, tag="hex")
                        he_v = he[:].rearrange("p (c l) -> p c l", l=EXT)
                        nc.gpsimd.memset(he_v[:, :, 128:129], 1.0)
                        xb = hcm.tile([P, SLC], BF16, tag="hcm")
                        nc.vector.tensor_copy(xb[:], xs_tiles[s][:])
                        for g in range(2):
                            xq = qps.tile([P, ZW], BF16, tag="qp")
                            for k in range(8):
                                c = 8 * g + k
                                nc.tensor.transpose(
                                    xq[:, 128 * k:128 * (k + 1)],
                                    xb[:, 128 * c:128 * (c + 1)],
                                    ident[:],
                                )
                            nc.vector.tensor_copy(
                                he_v[:, 8 * g:8 * (g + 1), 0:128],
                                xq[:].rearrange("p (c l) -> p c l", l=128),
                            )
                    else:
                        # filled by previous block's phase 2 (PE transposes)
                        he = he_next[s]
                        he_v = he[:].rearrange("p (c l) -> p c l", l=EXT)

                    # E = exp(h) (token-major, bf16, compact)
                    et = etm.tile([P, SLC], BF16, tag="etm")
                    nc.scalar.activation(et[:], he_v[:, :, 0:128], AF.Exp)
                    if DEBUG and blk == DBG_BLK and s == 0:
                        nc.sync.dma_start(dbg_he[:], he[:])
                        nc.sync.dma_start(dbg_et[:], et[:])
                    # gram + ksum: G[d, e] += E_c^T @ [h_c | 1]
                    for c in range(NCH):
                        nc.tensor.matmul(
                            G_ps[:],
                            et[:, 128 * c:128 * (c + 1)],
                            he_v[:, c, :],
                            start=(s == 0 and c == 0),
                            stop=(s == NSL - 1 and c == NCH - 1),
                        )
                    # q-softmax denominator: segmented sum over d (free dim)
                    sq = sqp.tile([P, NCH * 16], F32, tag="sq")
                    nc.vector.reduce_sum(
                        sq[:],
                        et[:].rearrange("p (c g d) -> p c g d", g=16, d=DH),
                        axis=mybir.AxisListType.X,
                    )
                    rq = sqp.tile([P, NCH * 16], F32, tag="rq")
                    nc.vector.reciprocal(rq[:], sq[:])
                    # q = E * (1/sq) broadcast over d  (Pool)
                    qt = qtm.tile([P, SLC], BF16, tag="qtm")
                    nc.gpsimd.tensor_tensor(
                        qt[:].rearrange("p (c g d) -> p c g d", g=16, d=DH),
                        et[:].rearrange("p (c g d) -> p c g d", g=16, d=DH),
                        rq[:].rearrange("p (c g) -> p c g", g=16)
                            .unsqueeze(-1).broadcast_to([P, NCH, 16, DH]),
                        op=MUL,
                    )
                    # transpose q to channel-major via PE
                    for g in range(2):
                        qp = qps.tile([P, ZW], BF16, tag="qp")
                        for k in range(8):
                            c = 8 * g + k
                            nc.tensor.transpose(
                                qp[:, 128 * k:128 * (k + 1)],
                                qt[:, 128 * c:128 * (c + 1)],
                                ident[:],
                            )
                        nc.vector.tensor_copy(
                            q_cm[:, SLC * s + ZW * g: SLC * s + ZW * (g + 1)],
                            qp[:],
                        )

                # ===================== M1 build =========================
                ksC = smal.tile([P, 1], F32, tag="ksC")
                nc.vector.reciprocal(ksC[:], G_ps[:, 128:129])
                G_sb = smal.tile([P, P], BF16, tag="Gsb")
                nc.vector.tensor_tensor(G_sb[:], G_ps[:, 0:128], headmask[:],
                                        op=MUL)
                GT2_ps = tps.tile([P, C], BF16, tag="tiny")
                for b in range(B_LOC):
                    sl = slice(C * b, C * (b + 1))
                    nc.tensor.transpose(
                        GT2_ps[sl, 0:C], G_sb[sl, sl], ident[sl, sl],
                        tile_position=(C * b, C * b),
                    )
                GT2_sb = smal.tile([P, C], BF16, tag="gt2sb")
                nc.vector.tensor_copy(GT2_sb[:], GT2_ps[:])
                M1u_ps = tps.tile([P, C], F32, tag="tiny")
                for b in range(B_LOC):
                    sl = slice(C * b, C * (b + 1))
                    nc.tensor.matmul(
                        M1u_ps[sl, 0:C], GT2_sb[sl, :], W1rep[blk][sl, :],
                        tile_position=(C * b, C * b),
                    )
                # M1blk = diag(1/ksum) @ G @ W1 scattered block-diagonally
                M1blk = m1p.tile([P, P], BF16, tag="m1b")
                nc.gpsimd.memset(M1blk[:], 0.0)
                for b in range(B_LOC):
                    sl = slice(C * b, C * (b + 1))
                    nc.vector.tensor_scalar_mul(
                        M1blk[sl, C * b:C * (b + 1)], M1u_ps[sl, 0:C],
                        ksC[sl, :])

                if DEBUG and blk == DBG_BLK:
                    gtmp = smal.tile([P, EXT], F32, tag="gdump")
                    nc.vector.tensor_copy(gtmp[:], G_ps[:])
                    nc.sync.dma_start(dbg_G[:], gtmp[:])
                    nc.sync.dma_start(dbg_ksC[:], ksC[:])
                    nc.sync.dma_start(dbg_M1[:], M1blk[:])
                # ===================== phase 2 (channel-major) ==========
                last = blk == NB - 1
                if not last:
                    he_next = []
                    for s in range(NSL):
                        hx = hex_.tile([P, NCH * EXT], BF16, tag="hex")
                        nc.gpsimd.memset(
                            hx[:].rearrange("p (c l) -> p c l", l=EXT)
                            [:, :, 128:129], 1.0)
                        he_next.append(hx)
                for t in range(NZ):
                    z1 = zps.tile([P, ZW], F32, tag="z")
                    for hw in range(2):
                        cs = ZW * t + 512 * hw
                        nc.tensor.matmul(
                            z1[:, 512 * hw:512 * (hw + 1)], M1blk[:],
                            q_cm[:, cs:cs + 512],
                        )
                    a1 = a1p.tile([P, ZW], BF16, tag="a1")
                    nc.scalar.activation(a1[:], z1[:], AF.Gelu,
                                         bias=b1rep[blk][:])
                    z2 = zps.tile([P, ZW], F32, tag="z")
                    for hw in range(2):
                        nc.tensor.matmul(
                            z2[:, 512 * hw:512 * (hw + 1)], W2blk[blk][:],
                            a1[:, 512 * hw:512 * (hw + 1)],
                        )
                    if t % 2 == 0:
                        hn = hcm.tile([P, SLC], BF16, tag="hcm")
                    ho = hn[:, ZW * (t % 2):ZW * (t % 2 + 1)]
                    if last:
                        nc.scalar.activation(
                            ho, z2[:], AF.Gelu, bias=b2rep[blk][:],
                            accum_out=pooled_parts[:, t:t + 1],
                        )
                    else:
                        nc.scalar.activation(
                            ho, z2[:], AF.Gelu, bias=b2rep[blk][:],
                        )
                    if DEBUG and blk == 0 and t == 0:
                        nc.sync.dma_start(dbg_q[:], q_cm[:, 0:SLC])
                    if DEBUG and blk == 0 and t == 1:
                        nc.sync.dma_start(dbg_h1[:], hn[:])
                    if t % 2 == 1 and not last:
                        # next block's token-major h via PE transposes
                        s = t // 2
                        hx_v = he_next[s][:].rearrange(
                            "p (c l) -> p c l", l=EXT)
                        for g in range(2):
                            hq = qps.tile([P, ZW], BF16, tag="qp")
                            for k in range(8):
                                c = 8 * g + k
                                nc.tensor.transpose(
                                    hq[:, 128 * k:128 * (k + 1)],
                                    hn[:, 128 * c:128 * (c + 1)],
                                    ident[:],
                                )
                            nc.vector.tensor_copy(
                                hx_v[:, 8 * g:8 * (g + 1), 0:128],
                                hq[:].rearrange("p (c l) -> p c l", l=128),
                            )

            # ===================== head =============================
            psum_ = smal.tile([P, 1], F32, tag="poolsum")
            nc.vector.reduce_sum(psum_[:], pooled_parts[:],
                                 axis=mybir.AxisListType.X)
            y_ps = tps.tile([P, C], F32, tag="tiny")
            for b in range(B_LOC):
                sl = slice(C * b, C * (b + 1))
                nc.tensor.matmul(
                    y_ps[sl, 0:1], Whrep[sl, :], psum_[sl, :],
                    tile_position=(C * b, C * b),
                )
            ybn = smal.tile([P, 1], F32, tag="ybn")
            nc.vector.tensor_scalar(
                ybn[:], y_ps[:, 0:1], svecL[:], tvec[:], op0=MUL, op1=ADD,
            )
            yg = smal.tile([P, 1], F32, tag="yg")
            nc.scalar.activation(yg[:], ybn[:], AF.Gelu)
            o_ps = tps.tile([P, C], F32, tag="tiny")
            for b in range(B_LOC):
                nc.tensor.matmul(
                    o_ps[C * b:C * b + DOUT, 0:1],
                    Wfrep[C * b:C * (b + 1), :],
                    yg[C * b:C * (b + 1), :],
                    tile_position=(C * b, C * b),
                )
            ob = smal.tile([P, 1], F32, tag="ob")
            for b in range(B_LOC):
                sl = slice(C * b, C * b + DOUT)
                nc.vector.tensor_tensor(ob[sl, :], o_ps[sl, 0:1], bf_s[sl, :],
                                        op=ADD)
            for b in range(B_LOC):
                nc.sync.dma_start(
                    out_d[b, :], ob[C * b:C * b + DOUT, 0],
                )

    _fix_sync_waits(nc)
    return nc


_NC_CACHE = [None]


def kernel(**inputs) -> np.ndarray:
    arrs = {k: np.asarray(v, dtype=np.float32) for k, v in inputs.items()}
    x = arrs["x"]
    B = x.shape[0]
    n_cores = 8
    bl = B // n_cores

    if _NC_CACHE[0] is None:
        _NC_CACHE[0] = build_program()
    nc = _NC_CACHE[0]

    params = {k: arrs[k] for k in (
        "fcW1", "fcb1", "fcW2", "fcb2", "Wh", "bh",
        "bn_gamma", "bn_beta", "bn_mean", "bn_var", "Wf", "bf")}
    in_maps = [
        {"x": np.ascontiguousarray(x[bl * i: bl * (i + 1)]), **params}
        for i in range(n_cores)
    ]
    res = run_bass_kernel_spmd(nc, in_maps, list(range(n_cores))).results
    return np.concatenate([res[i]["out"] for i in range(n_cores)], axis=0)


# revision 16
# speedup vs baseline: 1.1249x; 1.1249x over previous
"""Trainium2 Bass kernel for nn_CLFormer (3-block linear-attention transformer).

Sharding: pure data parallel — batch 32 split as 4 per NeuronCore across 8
cores; all parameters replicated; outputs concatenated.

Per-core layout: 4 batches x 32 channels packed on the 128 SBUF partitions
("channel-major" [128=4bx32c, L]). The kv-gram contracts over tokens, so a
token-major copy is produced per block by PE transposes (block 0 from a
bf16 cast of x loaded via casting DMA; blocks 1-2 from the gelu output
during the previous phase 2 — no DRAM round trip). The token-major value
operand carries a ones column (129-wide chunks) so the gram matmul also
produces the k-softmax denominator for free. Attention-out and FC1 fuse
into M1 = diag(1/ksum) @ G @ W1; FC matmuls run as single 128-wide matmuls
with block-diagonal weights. Replicated/derived parameters (weight strips,
BN eval folding) are precomputed on the host.
"""
import sys
import numpy as np

for _p in ("/opt/trn_rl_repo", "/root/.axon_site/_ro/trn_rl_repo"):
    if _p not in sys.path:
        sys.path.append(_p)

from contextlib import ExitStack

import concourse.bass as bass
import concourse.mybir as mybir
import bass_rust
from concourse import tile
from concourse.masks import make_identity
from concourse.bass_utils import run_bass_kernel_spmd

F32 = mybir.dt.float32
BF16 = mybir.dt.bfloat16
AF = mybir.ActivationFunctionType
MUL = mybir.AluOpType.mult
ADD = mybir.AluOpType.add

P = 128
B_LOC = 4            # batches per core
C = 32               # channels
L = 16384            # sequence length
NB = 3               # transformer blocks
DOUT = 10
HEADS = 4
DH = 8
BN_EPS = 1e-5

SLC = 2048           # slice width (tokens per pipeline slice)
NSL = L // SLC       # 8 slices
NCH = SLC // 128     # 16 chunks per slice
EXT = 129            # chunk width in the ones-extended token-major h tile
ZW = 1024            # phase-2 tile width (tokens per z/gelu tile)
NZ = L // ZW         # 16 phase-2 steps
DEBUG = False
DBG_BLK = 0


# ---------------------------------------------------------------- waitfix --
_WF_SKIP = {"InstEventSemaphore"}
_wf_ctr = [0]


def _fix_sync_waits(nc):
    """Hoist excess sync waits onto InstEventSemaphore (this walrus build
    accepts only 1 wait per instruction). The event-sem executes on the same
    engine stream immediately before, preserving semantics."""
    for fn in nc.m.functions:
        new_blocks = []
        for blk in fn.blocks:
            out = []
            for ins in blk.instructions:
                tname = type(ins).__name__
                si = ins.sync_info
                if si is None or tname in _WF_SKIP:
                    out.append(ins)
                    continue
                waits = list(si.on_wait)
                if len(waits) <= 1:
                    out.append(ins)
                    continue
                keep = waits[-1:]
                excess = waits[:-1]
                for i in range(0, len(excess), 2):
                    chunk = excess[i:i + 2]
                    _wf_ctr[0] += 1
                    ev = mybir.InstEventSemaphore(
                        name=f"wfix{_wf_ctr[0]}", ins=[], outs=[])
                    ev.engine = ins.engine
                    ev.sync_info = mybir.SyncInfo(on_wait=chunk, on_update=[])
                    out.append(ev)
                ins.sync_info = mybir.SyncInfo(
                    on_wait=keep, on_update=list(si.on_update))
                out.append(ins)
            nb = bass_rust.BasicBlock(name=blk.name, instructions=out)
            new_blocks.append(nb)
        fn.blocks = new_blocks


def _load_w(nc, pool, src_ap, cols, dtype, tag):
    """DRAM [128, cols] -> SBUF, single DMA (+ cast if bf16)."""
    stage = pool.tile([P, cols], F32, tag=f"{tag}_st")
    nc.sync.dma_start(stage[:], src_ap)
    if dtype == F32:
        return stage
    out = pool.tile([P, cols], dtype, tag=f"{tag}_bf")
    nc.vector.tensor_copy(out[:], stage[:])
    return out


# ---------------------------------------------------------------- program --
def build_program(reps=1):
    nc = bass.Bass()

    x_d = nc.declare_dram_parameter("x", [B_LOC, C, L], F32, isOutput=False)
    # host-precomputed replicated / derived parameters
    W1r_d = nc.declare_dram_parameter("W1r", [NB, P, C], F32, isOutput=False)
    W2b_d = nc.declare_dram_parameter("W2b", [NB, P, P], F32, isOutput=False)
    b1r_d = nc.declare_dram_parameter("b1r", [NB, P], F32, isOutput=False)
    b2r_d = nc.declare_dram_parameter("b2r", [NB, P], F32, isOutput=False)
    Whr_d = nc.declare_dram_parameter("Whr", [P, C], F32, isOutput=False)
    Wfr_d = nc.declare_dram_parameter("Wfr", [P, DOUT], F32, isOutput=False)
    svecL_d = nc.declare_dram_parameter("svecLr", [P], F32, isOutput=False)
    tvec_d = nc.declare_dram_parameter("tvecr", [P], F32, isOutput=False)
    bf_d = nc.declare_dram_parameter("bfr", [P], F32, isOutput=False)
    out_d = nc.declare_dram_parameter("out", [B_LOC, DOUT], F32, isOutput=True)
    if DEBUG:
        dbg_he = nc.declare_dram_parameter("dbg_he", [P, NCH * EXT], BF16, isOutput=True)
        dbg_et = nc.declare_dram_parameter("dbg_et", [P, SLC], BF16, isOutput=True)
        dbg_G = nc.declare_dram_parameter("dbg_G", [P, EXT], F32, isOutput=True)
        dbg_ksC = nc.declare_dram_parameter("dbg_ksC", [P, 1], F32, isOutput=True)
        dbg_M1 = nc.declare_dram_parameter("dbg_M1", [P, P], BF16, isOutput=True)
        dbg_q = nc.declare_dram_parameter("dbg_q", [P, SLC], BF16, isOutput=True)
        dbg_h1 = nc.declare_dram_parameter("dbg_h1", [P, SLC], BF16, isOutput=True)

    with ExitStack() as ctx:
        tc = ctx.enter_context(tile.TileContext(nc))
        cst = ctx.enter_context(tc.tile_pool(name="cst", bufs=1))
        hcm = ctx.enter_context(tc.tile_pool(name="hcm", bufs=3))
        hex_ = ctx.enter_context(tc.tile_pool(name="hex", bufs=8))
        etm = ctx.enter_context(tc.tile_pool(name="etm", bufs=3))
        qtm = ctx.enter_context(tc.tile_pool(name="qtm", bufs=3))
        sqp = ctx.enter_context(tc.tile_pool(name="sqp", bufs=3))
        bigq = ctx.enter_context(tc.tile_pool(name="bigq", bufs=2))
        a1p = ctx.enter_context(tc.tile_pool(name="a1p", bufs=3))
        smal = ctx.enter_context(tc.tile_pool(name="smal", bufs=2))
        m1p = ctx.enter_context(tc.tile_pool(name="m1p", bufs=2))
        gps = ctx.enter_context(tc.tile_pool(name="gps", bufs=1, space="PSUM"))
        zps = ctx.enter_context(tc.tile_pool(name="zps", bufs=2, space="PSUM"))
        qps = ctx.enter_context(tc.tile_pool(name="qps", bufs=2, space="PSUM"))
        tps = ctx.enter_context(tc.tile_pool(name="tps", bufs=1, space="PSUM"))

        for _rep in range(reps):
            x_cm = x_d[:].rearrange("b c l -> (b c) l")

            # ---- constants (issued first; all single DMAs on sync) -----
            if _rep == 0:
                ident = cst.tile([P, P], BF16)
                make_identity(nc, ident[:])
                headmask = cst.tile([P, P], BF16)
                nc.vector.memset(headmask[:], 1.0)
                hm_v = headmask[:].rearrange("p (g i) -> p g i", i=DH)
                nc.gpsimd.affine_select(
                    out=hm_v, in_=hm_v, pattern=[[-DH, P // DH], [0, DH]],
                    compare_op=mybir.AluOpType.is_ge, fill=0.0,
                    base=0, channel_multiplier=1)
                nc.gpsimd.affine_select(
                    out=hm_v, in_=hm_v, pattern=[[DH, P // DH], [0, DH]],
                    compare_op=mybir.AluOpType.is_ge, fill=0.0,
                    base=DH - 1, channel_multiplier=-1)

                W1rep = [_load_w(nc, cst, W1r_d[i], C, BF16, f"w1_{i}")
                         for i in range(NB)]
                W2blk = [_load_w(nc, cst, W2b_d[i], P, BF16, f"w2_{i}")
                         for i in range(NB)]
                b1rep = [_load_w(nc, cst, b1r_d[i].unsqueeze(-1), 1, F32,
                                 f"b1_{i}") for i in range(NB)]
                b2rep = [_load_w(nc, cst, b2r_d[i].unsqueeze(-1), 1, F32,
                                 f"b2_{i}") for i in range(NB)]
                Whrep = _load_w(nc, cst, Whr_d[:], C, F32, "wh")
                Wfrep = _load_w(nc, cst, Wfr_d[:], DOUT, F32, "wf")
                svecL = _load_w(nc, cst, svecL_d[:].unsqueeze(-1), 1, F32, "sv")
                tvec = _load_w(nc, cst, tvec_d[:].unsqueeze(-1), 1, F32, "tv")
                bf_s = _load_w(nc, cst, bf_d[:].unsqueeze(-1), 1, F32, "bf")

            pooled_parts = cst.tile([P, NZ], F32, tag="pool")

            # ------------- per-slice phase-1 stages -----------------------
            def stage_a0(s):
                """Block-0 input: casting DMA + PE transpose -> he tile."""
                he = hex_.tile([P, NCH * EXT], BF16, tag="hex")
                he_v = he[:].rearrange("p (c l) -> p c l", l=EXT)
                nc.gpsimd.memset(he_v[:, :, 128:129], 1.0)
                xb = hcm.tile([P, SLC], BF16, tag="hcm")
                nc.gpsimd.dma_start(xb[:], x_cm[:, SLC * s:SLC * (s + 1)])
                for g in range(2):
                    xq = qps.tile([P, ZW], BF16, tag="qp")
                    for k in range(8):
                        c = 8 * g + k
                        nc.tensor.transpose(
                            xq[:, 128 * k:128 * (k + 1)],
                            xb[:, 128 * c:128 * (c + 1)],
                            ident[:],
                        )
                    nc.vector.tensor_copy(
                        he_v[:, 8 * g:8 * (g + 1), 0:128],
                        xq[:].rearrange("p (c l) -> p c l", l=128),
                    )
                return he

            def stage_b(blk, s, he, G_ps, q_cm):
                """exp -> gram/ksum -> q scale -> q transpose."""
                he_v = he[:].rearrange("p (c l) -> p c l", l=EXT)
                et = etm.tile([P, SLC], BF16, tag="etm")
                nc.scalar.activation(et[:], he_v[:, :, 0:128], AF.Exp)
                if DEBUG and blk == DBG_BLK and s == 0:
                    nc.sync.dma_start(dbg_he[:], he[:])
                    nc.sync.dma_start(dbg_et[:], et[:])
                for c in range(NCH):
                    nc.tensor.matmul(
                        G_ps[:],
                        et[:, 128 * c:128 * (c + 1)],
                        he_v[:, c, :],
                        start=(s == 0 and c == 0),
                        stop=(s == NSL - 1 and c == NCH - 1),
                    )
                # q-softmax denominator: segmented sum over d (Pool)
                sq = sqp.tile([P, NCH * 16], F32, tag="sq")
                nc.vector.reduce_sum(
                    sq[:],
                    et[:].rearrange("p (c g d) -> p c g d", g=16, d=DH),
                    axis=mybir.AxisListType.X,
                )
                rq = sqp.tile([P, NCH * 16], F32, tag="rq")
                nc.vector.reciprocal(rq[:], sq[:])  # approx_fast: ISA err?
                # q = E * (1/sq) broadcast over d (alternate DVE / Pool)
                qt = qtm.tile([P, SLC], BF16, tag="qtm")
                nc.gpsimd.tensor_tensor(
                    qt[:].rearrange("p (c g d) -> p c g d", g=16, d=DH),
                    et[:].rearrange("p (c g d) -> p c g d", g=16, d=DH),
                    rq[:].rearrange("p (c g) -> p c g", g=16)
                        .unsqueeze(-1).broadcast_to([P, NCH, 16, DH]),
                    op=MUL,
                )
                # transpose q to channel-major via PE
                for g in range(2):
                    qp = qps.tile([P, ZW], BF16, tag="qp")
                    for k in range(8):
                        c = 8 * g + k
                        nc.tensor.transpose(
                            qp[:, 128 * k:128 * (k + 1)],
                            qt[:, 128 * c:128 * (c + 1)],
                            ident[:],
                        )
                    nc.vector.tensor_copy(
                        q_cm[:, SLC * s + ZW * g: SLC * s + ZW * (g + 1)],
                        qp[:],
                    )

            he_next = None
            for blk in range(NB):
                # ===================== phase 1 (token-major) ============
                G_ps = gps.tile([P, EXT], F32, tag="G")
                q_cm = bigq.tile([P, L], BF16, tag="qcm")
                if blk == 0:
                    # stage-skewed issue: he(s+1) production before slice-s
                    # consumption so every engine queue overlaps slices
                    he_tiles = [stage_a0(0), stage_a0(1)]
                    for s in range(NSL):
                        if s + 2 < NSL:
                            he_tiles.append(stage_a0(s + 2))
                        stage_b(blk, s, he_tiles[s], G_ps, q_cm)
                else:
                    for s in range(NSL):
                        stage_b(blk, s, he_next[s], G_ps, q_cm)

                # ===================== M1 build =========================
                ksC = smal.tile([P, 1], F32, tag="ksC")
                nc.vector.reciprocal(ksC[:], G_ps[:, 128:129])
                G_sb = smal.tile([P, P], BF16, tag="Gsb")
                nc.vector.tensor_tensor(G_sb[:], G_ps[:, 0:128], headmask[:],
                                        op=MUL)
                GT2_ps = tps.tile([P, C], BF16, tag="tiny")
                for b in range(B_LOC):
                    sl = slice(C * b, C * (b + 1))
                    nc.tensor.transpose(
                        GT2_ps[sl, 0:C], G_sb[sl, sl], ident[sl, sl],
                        tile_position=(C * b, C * b),
                    )
                GT2_sb = smal.tile([P, C], BF16, tag="gt2sb")
                nc.vector.tensor_copy(GT2_sb[:], GT2_ps[:])
                M1u_ps = tps.tile([P, C], F32, tag="tiny")
                for b in range(B_LOC):
                    sl = slice(C * b, C * (b + 1))
                    nc.tensor.matmul(
                        M1u_ps[sl, 0:C], GT2_sb[sl, :], W1rep[blk][sl, :],
                        tile_position=(C * b, C * b),
                    )
                # M1blk = diag(1/ksum) @ G @ W1 scattered block-diagonally
                M1blk = m1p.tile([P, P], BF16, tag="m1b")
                nc.gpsimd.memset(M1blk[:], 0.0)
                for b in range(B_LOC):
                    sl = slice(C * b, C * (b + 1))
                    nc.vector.tensor_scalar_mul(
                        M1blk[sl, C * b:C * (b + 1)], M1u_ps[sl, 0:C],
                        ksC[sl, :])

                if DEBUG and blk == DBG_BLK:
                    gtmp = smal.tile([P, EXT], F32, tag="gdump")
                    nc.vector.tensor_copy(gtmp[:], G_ps[:])
                    nc.sync.dma_start(dbg_G[:], gtmp[:])
                    nc.sync.dma_start(dbg_ksC[:], ksC[:])
                    nc.sync.dma_start(dbg_M1[:], M1blk[:])
                # ===================== phase 2 (channel-major) ==========
                last = blk == NB - 1
                if not last:
                    he_next = []
                    for s in range(NSL):
                        hx = hex_.tile([P, NCH * EXT], BF16, tag="hex")
                        nc.gpsimd.memset(
                            hx[:].rearrange("p (c l) -> p c l", l=EXT)
                            [:, :, 128:129], 1.0)
                        he_next.append(hx)
                for t in range(NZ):
                    z1 = zps.tile([P, ZW], F32, tag="z")
                    for hw in range(2):
                        cs = ZW * t + 512 * hw
                        nc.tensor.matmul(
                            z1[:, 512 * hw:512 * (hw + 1)], M1blk[:],
                            q_cm[:, cs:cs + 512],
                        )
                    a1 = a1p.tile([P, ZW], BF16, tag="a1")
                    nc.scalar.activation(a1[:], z1[:], AF.Gelu,
                                         bias=b1rep[blk][:])
                    z2 = zps.tile([P, ZW], F32, tag="z")
                    for hw in range(2):
                        nc.tensor.matmul(
                            z2[:, 512 * hw:512 * (hw + 1)], W2blk[blk][:],
                            a1[:, 512 * hw:512 * (hw + 1)],
                        )
                    if t % 2 == 0:
                        hn = hcm.tile([P, SLC], BF16, tag="hcm")
                    ho = hn[:, ZW * (t % 2):ZW * (t % 2 + 1)]
                    if last:
                        nc.scalar.activation(
                            ho, z2[:], AF.Gelu, bias=b2rep[blk][:],
                            accum_out=pooled_parts[:, t:t + 1],
                        )
                    else:
                        nc.scalar.activation(
                            ho, z2[:], AF.Gelu, bias=b2rep[blk][:],
                        )
                    if DEBUG and blk == 0 and t == 0:
                        nc.sync.dma_start(dbg_q[:], q_cm[:, 0:SLC])
                    if DEBUG and blk == 0 and t == 1:
                        nc.sync.dma_start(dbg_h1[:], hn[:])
                    if t % 2 == 1 and not last:
                        # next block's token-major h via PE transposes
                        s = t // 2
                        hx_v = he_next[s][:].rearrange(
                            "p (c l) -> p c l", l=EXT)
                        for g in range(2):
                            hq = qps.tile([P, ZW], BF16, tag="qp")
                            for k in range(8):
                                c = 8 * g + k
                                nc.tensor.transpose(
                                    hq[:, 128 * k:128 * (k + 1)],
                                    hn[:, 128 * c:128 * (c + 1)],
                                    ident[:],
                                )
                            nc.vector.tensor_copy(
                                hx_v[:, 8 * g:8 * (g + 1), 0:128],
                                hq[:].rearrange("p (c l) -> p c l", l=128),
                            )

            # ===================== head =============================
            psum_ = smal.tile([P, 1], F32, tag="poolsum")
            nc.vector.reduce_sum(psum_[:], pooled_parts[:],
                                 axis=mybir.AxisListType.X)
            y_ps = tps.tile([P, C], F32, tag="tiny")
            for b in range(B_LOC):
                sl = slice(C * b, C * (b + 1))
                nc.tensor.matmul(
                    y_ps[sl, 0:1], Whrep[sl, :], psum_[sl, :],
                    tile_position=(C * b, C * b),
                )
            ybn = smal.tile([P, 1], F32, tag="ybn")
            nc.vector.tensor_scalar(
                ybn[:], y_ps[:, 0:1], svecL[:], tvec[:], op0=MUL, op1=ADD,
            )
            yg = smal.tile([P, 1], F32, tag="yg")
            nc.scalar.activation(yg[:], ybn[:], AF.Gelu)
            o_ps = tps.tile([P, C], F32, tag="tiny")
            for b in range(B_LOC):
                nc.tensor.matmul(
                    o_ps[C * b:C * b + DOUT, 0:1],
                    Wfrep[C * b:C * (b + 1), :],
                    yg[C * b:C * (b + 1), :],
                    tile_position=(C * b, C * b),
                )
            ob = smal.tile([P, 1], F32, tag="ob")
            for b in range(B_LOC):
                sl = slice(C * b, C * b + DOUT)
                nc.vector.tensor_tensor(ob[sl, :], o_ps[sl, 0:1], bf_s[sl, :],
                                        op=ADD)
            for b in range(B_LOC):
                nc.sync.dma_start(
                    out_d[b, :], ob[C * b:C * b + DOUT, 0],
                )

    _fix_sync_waits(nc)
    return nc


def _derive_params(arrs):
    """Host-side precompute: replicated weight strips + BN eval folding."""
    tile4 = lambda a: np.tile(a, (B_LOC,) + (1,) * (a.ndim - 1))
    W1 = arrs["fcW1"]            # [NB, C, C]
    W2 = arrs["fcW2"]
    W2b = np.zeros((NB, P, P), np.float32)
    for i in range(NB):
        for b in range(B_LOC):
            W2b[i, C * b:C * (b + 1), C * b:C * (b + 1)] = W2[i]
    svec = arrs["bn_gamma"] / np.sqrt(arrs["bn_var"] + BN_EPS)
    tv = (arrs["bh"] - arrs["bn_mean"]) * svec + arrs["bn_beta"]
    bfr = np.zeros(P, np.float32)
    for b in range(B_LOC):
        bfr[C * b:C * b + DOUT] = arrs["bf"]
    return {
        "W1r": np.ascontiguousarray(
            np.stack([tile4(W1[i]) for i in range(NB)])),
        "W2b": W2b,
        "b1r": np.ascontiguousarray(
            np.stack([tile4(arrs["fcb1"][i]) for i in range(NB)])),
        "b2r": np.ascontiguousarray(
            np.stack([tile4(arrs["fcb2"][i]) for i in range(NB)])),
        "Whr": tile4(arrs["Wh"]),
        "Wfr": tile4(arrs["Wf"]),
        "svecLr": tile4(svec / L),
        "tvecr": tile4(tv),
        "bfr": bfr,
    }


_NC_CACHE = [None]


def kernel(**inputs) -> np.ndarray:
    arrs = {k: np.asarray(v, dtype=np.float32) for k, v in inputs.items()}
    x = arrs["x"]
    B = x.shape[0]
    n_cores = 8
    bl = B // n_cores

    if _NC_CACHE[0] is None:
        _NC_CACHE[0] = build_program()
    nc = _NC_CACHE[0]

    params = _derive_params(arrs)
    in_maps = [
        {"x": np.ascontiguousarray(x[bl * i: bl * (i + 1)]), **params}
        for i in range(n_cores)
    ]
    res = run_bass_kernel_spmd(nc, in_maps, list(range(n_cores))).results
    return np.concatenate([res[i]["out"] for i in range(n_cores)], axis=0)


# revision 18
# speedup vs baseline: 1.1274x; 1.0022x over previous
"""Trainium2 Bass kernel for nn_CLFormer (3-block linear-attention transformer).

Sharding: pure data parallel — batch 32 split as 4 per NeuronCore across 8
cores; all parameters replicated; outputs concatenated.

Per-core layout: 4 batches x 32 channels packed on the 128 SBUF partitions
("channel-major" [128=4bx32c, L]). The kv-gram contracts over tokens, so a
token-major copy is produced per block by PE transposes (block 0 from a
bf16 cast of x loaded via casting DMA; blocks 1-2 from the gelu output
during the previous phase 2 — no DRAM round trip). The token-major value
operand carries a ones column (129-wide chunks) so the gram matmul also
produces the k-softmax denominator for free. Attention-out and FC1 fuse
into M1 = diag(1/ksum) @ G @ W1; FC matmuls run as single 128-wide matmuls
with block-diagonal weights. Replicated/derived parameters (weight strips,
BN eval folding) are precomputed on the host.
"""
import sys
import numpy as np

for _p in ("/opt/trn_rl_repo", "/root/.axon_site/_ro/trn_rl_repo"):
    if _p not in sys.path:
        sys.path.append(_p)

from contextlib import ExitStack

import concourse.bass as bass
import concourse.mybir as mybir
import bass_rust
from concourse import tile
from concourse.masks import make_identity
from concourse.bass_utils import run_bass_kernel_spmd

F32 = mybir.dt.float32
BF16 = mybir.dt.bfloat16
AF = mybir.ActivationFunctionType
MUL = mybir.AluOpType.mult
ADD = mybir.AluOpType.add

P = 128
B_LOC = 4            # batches per core
C = 32               # channels
L = 16384            # sequence length
NB = 3               # transformer blocks
DOUT = 10
HEADS = 4
DH = 8
BN_EPS = 1e-5

SLC = 2048           # slice width (tokens per pipeline slice)
NSL = L // SLC       # 8 slices
NCH = SLC // 128     # 16 chunks per slice
EXT = 129            # chunk width in the ones-extended token-major h tile
ZW = 1024            # phase-2 tile width (tokens per z/gelu tile)
NZ = L // ZW         # 16 phase-2 steps
DEBUG = False
DBG_BLK = 0


# ---------------------------------------------------------------- waitfix --
_WF_SKIP = {"InstEventSemaphore"}
_wf_ctr = [0]


def _fix_sync_waits(nc):
    """Hoist excess sync waits onto InstEventSemaphore (this walrus build
    accepts only 1 wait per instruction). The event-sem executes on the same
    engine stream immediately before, preserving semantics."""
    for fn in nc.m.functions:
        new_blocks = []
        for blk in fn.blocks:
            out = []
            for ins in blk.instructions:
                tname = type(ins).__name__
                si = ins.sync_info
                if si is None or tname in _WF_SKIP:
                    out.append(ins)
                    continue
                waits = list(si.on_wait)
                if len(waits) <= 1:
                    out.append(ins)
                    continue
                keep = waits[-1:]
                excess = waits[:-1]
                for i in range(0, len(excess), 2):
                    chunk = excess[i:i + 2]
                    _wf_ctr[0] += 1
                    ev = mybir.InstEventSemaphore(
                        name=f"wfix{_wf_ctr[0]}", ins=[], outs=[])
                    ev.engine = ins.engine
                    ev.sync_info = mybir.SyncInfo(on_wait=chunk, on_update=[])
                    out.append(ev)
                ins.sync_info = mybir.SyncInfo(
                    on_wait=keep, on_update=list(si.on_update))
                out.append(ins)
            nb = bass_rust.BasicBlock(name=blk.name, instructions=out)
            new_blocks.append(nb)
        fn.blocks = new_blocks


def _load_w(nc, pool, src_ap, cols, dtype, tag):
    """DRAM [128, cols] -> SBUF, single DMA (+ cast if bf16)."""
    stage = pool.tile([P, cols], F32, tag=f"{tag}_st")
    nc.sync.dma_start(stage[:], src_ap)
    if dtype == F32:
        return stage
    out = pool.tile([P, cols], dtype, tag=f"{tag}_bf")
    nc.vector.tensor_copy(out[:], stage[:])
    return out


# ---------------------------------------------------------------- program --
def build_program(reps=1):
    nc = bass.Bass()

    x_d = nc.declare_dram_parameter("x", [B_LOC, C, L], F32, isOutput=False)
    # host-precomputed replicated / derived parameters
    W1r_d = nc.declare_dram_parameter("W1r", [NB, P, C], F32, isOutput=False)
    W2b_d = nc.declare_dram_parameter("W2b", [NB, P, P], F32, isOutput=False)
    b1r_d = nc.declare_dram_parameter("b1r", [NB, P], F32, isOutput=False)
    b2r_d = nc.declare_dram_parameter("b2r", [NB, P], F32, isOutput=False)
    Whr_d = nc.declare_dram_parameter("Whr", [P, C], F32, isOutput=False)
    Wfr_d = nc.declare_dram_parameter("Wfr", [P, DOUT], F32, isOutput=False)
    svecL_d = nc.declare_dram_parameter("svecLr", [P], F32, isOutput=False)
    tvec_d = nc.declare_dram_parameter("tvecr", [P], F32, isOutput=False)
    bf_d = nc.declare_dram_parameter("bfr", [P], F32, isOutput=False)
    out_d = nc.declare_dram_parameter("out", [B_LOC, DOUT], F32, isOutput=True)
    if DEBUG:
        dbg_he = nc.declare_dram_parameter("dbg_he", [P, NCH * EXT], BF16, isOutput=True)
        dbg_et = nc.declare_dram_parameter("dbg_et", [P, SLC], BF16, isOutput=True)
        dbg_G = nc.declare_dram_parameter("dbg_G", [P, EXT], F32, isOutput=True)
        dbg_ksC = nc.declare_dram_parameter("dbg_ksC", [P, 1], F32, isOutput=True)
        dbg_M1 = nc.declare_dram_parameter("dbg_M1", [P, P], BF16, isOutput=True)
        dbg_q = nc.declare_dram_parameter("dbg_q", [P, SLC], BF16, isOutput=True)
        dbg_h1 = nc.declare_dram_parameter("dbg_h1", [P, SLC], BF16, isOutput=True)

    with ExitStack() as ctx:
        tc = ctx.enter_context(tile.TileContext(nc))
        cst = ctx.enter_context(tc.tile_pool(name="cst", bufs=1))
        hcm = ctx.enter_context(tc.tile_pool(name="hcm", bufs=3))
        hex_ = ctx.enter_context(tc.tile_pool(name="hex", bufs=8))
        etm = ctx.enter_context(tc.tile_pool(name="etm", bufs=3))
        qtm = ctx.enter_context(tc.tile_pool(name="qtm", bufs=3))
        sqp = ctx.enter_context(tc.tile_pool(name="sqp", bufs=3))
        bigq = ctx.enter_context(tc.tile_pool(name="bigq", bufs=2))
        a1p = ctx.enter_context(tc.tile_pool(name="a1p", bufs=3))
        smal = ctx.enter_context(tc.tile_pool(name="smal", bufs=2))
        m1p = ctx.enter_context(tc.tile_pool(name="m1p", bufs=2))
        gps = ctx.enter_context(tc.tile_pool(name="gps", bufs=1, space="PSUM"))
        zps = ctx.enter_context(tc.tile_pool(name="zps", bufs=2, space="PSUM"))
        qps = ctx.enter_context(tc.tile_pool(name="qps", bufs=2, space="PSUM"))
        tps = ctx.enter_context(tc.tile_pool(name="tps", bufs=1, space="PSUM"))

        for _rep in range(reps):
            x_cm = x_d[:].rearrange("b c l -> (b c) l")

            # ---- constants (issued first; all single DMAs on sync) -----
            if _rep == 0:
                ident = cst.tile([P, P], BF16)
                make_identity(nc, ident[:])
                headmask = cst.tile([P, P], BF16)
                nc.vector.memset(headmask[:], 1.0)
                hm_v = headmask[:].rearrange("p (g i) -> p g i", i=DH)
                nc.gpsimd.affine_select(
                    out=hm_v, in_=hm_v, pattern=[[-DH, P // DH], [0, DH]],
                    compare_op=mybir.AluOpType.is_ge, fill=0.0,
                    base=0, channel_multiplier=1)
                nc.gpsimd.affine_select(
                    out=hm_v, in_=hm_v, pattern=[[DH, P // DH], [0, DH]],
                    compare_op=mybir.AluOpType.is_ge, fill=0.0,
                    base=DH - 1, channel_multiplier=-1)

                W1rep = [_load_w(nc, cst, W1r_d[i], C, BF16, f"w1_{i}")
                         for i in range(NB)]
                W2blk = [_load_w(nc, cst, W2b_d[i], P, BF16, f"w2_{i}")
                         for i in range(NB)]
                b1rep = [_load_w(nc, cst, b1r_d[i].unsqueeze(-1), 1, F32,
                                 f"b1_{i}") for i in range(NB)]
                b2rep = [_load_w(nc, cst, b2r_d[i].unsqueeze(-1), 1, F32,
                                 f"b2_{i}") for i in range(NB)]
                Whrep = _load_w(nc, cst, Whr_d[:], C, F32, "wh")
                Wfrep = _load_w(nc, cst, Wfr_d[:], DOUT, F32, "wf")
                svecL = _load_w(nc, cst, svecL_d[:].unsqueeze(-1), 1, F32, "sv")
                tvec = _load_w(nc, cst, tvec_d[:].unsqueeze(-1), 1, F32, "tv")
                bf_s = _load_w(nc, cst, bf_d[:].unsqueeze(-1), 1, F32, "bf")

            pooled_parts = cst.tile([P, NZ], F32, tag="pool")

            # ------------- per-slice phase-1 stages -----------------------
            def stage_a0(s):
                """Block-0 input: casting DMA + PE transpose -> he tile."""
                he = hex_.tile([P, NCH * EXT], BF16, tag="hex")
                he_v = he[:].rearrange("p (c l) -> p c l", l=EXT)
                nc.gpsimd.memset(he_v[:, :, 128:129], 1.0)
                xb = hcm.tile([P, SLC], BF16, tag="hcm")
                nc.gpsimd.dma_start(xb[:], x_cm[:, SLC * s:SLC * (s + 1)])
                for g in range(2):
                    xq = qps.tile([P, ZW], BF16, tag="qp")
                    for k in range(8):
                        c = 8 * g + k
                        nc.tensor.transpose(
                            xq[:, 128 * k:128 * (k + 1)],
                            xb[:, 128 * c:128 * (c + 1)],
                            ident[:],
                        )
                    nc.vector.tensor_copy(
                        he_v[:, 8 * g:8 * (g + 1), 0:128],
                        xq[:].rearrange("p (c l) -> p c l", l=128),
                    )
                return he

            def stage_b(blk, s, he, G_ps, q_cm):
                """exp -> gram/ksum -> q scale -> q transpose."""
                he_v = he[:].rearrange("p (c l) -> p c l", l=EXT)
                et = etm.tile([P, SLC], BF16, tag="etm")
                nc.scalar.activation(et[:], he_v[:, :, 0:128], AF.Exp)
                if DEBUG and blk == DBG_BLK and s == 0:
                    nc.sync.dma_start(dbg_he[:], he[:])
                    nc.sync.dma_start(dbg_et[:], et[:])
                for c in range(NCH):
                    nc.tensor.matmul(
                        G_ps[:],
                        et[:, 128 * c:128 * (c + 1)],
                        he_v[:, c, :],
                        start=(s == 0 and c == 0),
                        stop=(s == NSL - 1 and c == NCH - 1),
                    )
                # q-softmax denominator: segmented sum over d (Pool)
                sq = sqp.tile([P, NCH * 16], F32, tag="sq")
                nc.vector.reduce_sum(
                    sq[:],
                    et[:].rearrange("p (c g d) -> p c g d", g=16, d=DH),
                    axis=mybir.AxisListType.X,
                )
                rq = sqp.tile([P, NCH * 16], F32, tag="rq")
                nc.vector.reciprocal(rq[:], sq[:])
                # q = E * (1/sq) broadcast over d (Pool)
                qt = qtm.tile([P, SLC], BF16, tag="qtm")
                nc.gpsimd.tensor_tensor(
                    qt[:].rearrange("p (c g d) -> p c g d", g=16, d=DH),
                    et[:].rearrange("p (c g d) -> p c g d", g=16, d=DH),
                    rq[:].rearrange("p (c g) -> p c g", g=16)
                        .unsqueeze(-1).broadcast_to([P, NCH, 16, DH]),
                    op=MUL,
                )
                # transpose q to channel-major via PE
                for g in range(2):
                    qp = qps.tile([P, ZW], BF16, tag="qp")
                    for k in range(8):
                        c = 8 * g + k
                        nc.tensor.transpose(
                            qp[:, 128 * k:128 * (k + 1)],
                            qt[:, 128 * c:128 * (c + 1)],
                            ident[:],
                        )
                    nc.vector.tensor_copy(
                        q_cm[:, SLC * s + ZW * g: SLC * s + ZW * (g + 1)],
                        qp[:],
                    )

            he_next = None
            for blk in range(NB):
                # ===================== phase 1 (token-major) ============
                G_ps = gps.tile([P, EXT], F32, tag="G")
                q_cm = bigq.tile([P, L], BF16, tag="qcm")
                if blk == 0:
                    # stage-skewed issue: he(s+1) production before slice-s
                    # consumption so every engine queue overlaps slices
                    he_tiles = [stage_a0(0), stage_a0(1)]
                    for s in range(NSL):
                        if s + 2 < NSL:
                            he_tiles.append(stage_a0(s + 2))
                        stage_b(blk, s, he_tiles[s], G_ps, q_cm)
                else:
                    for s in range(NSL):
                        stage_b(blk, s, he_next[s], G_ps, q_cm)

                # ===================== M1 build =========================
                ksC = smal.tile([P, 1], F32, tag="ksC")
                nc.vector.reciprocal(ksC[:], G_ps[:, 128:129])
                G_sb = smal.tile([P, P], BF16, tag="Gsb")
                nc.vector.tensor_tensor(G_sb[:], G_ps[:, 0:128], headmask[:],
                                        op=MUL)
                GT2_ps = tps.tile([P, C], BF16, tag="tiny")
                for b in range(B_LOC):
                    sl = slice(C * b, C * (b + 1))
                    nc.tensor.transpose(
                        GT2_ps[sl, 0:C], G_sb[sl, sl], ident[sl, sl],
                        tile_position=(C * b, C * b),
                    )
                GT2_sb = smal.tile([P, C], BF16, tag="gt2sb")
                nc.vector.tensor_copy(GT2_sb[:], GT2_ps[:])
                M1u_ps = tps.tile([P, C], F32, tag="tiny")
                for b in range(B_LOC):
                    sl = slice(C * b, C * (b + 1))
                    nc.tensor.matmul(
                        M1u_ps[sl, 0:C], GT2_sb[sl, :], W1rep[blk][sl, :],
                        tile_position=(C * b, C * b),
                    )
                # M1blk = diag(1/ksum) @ G @ W1 scattered block-diagonally
                M1blk = m1p.tile([P, P], BF16, tag="m1b")
                nc.gpsimd.memset(M1blk[:], 0.0)
                for b in range(B_LOC):
                    sl = slice(C * b, C * (b + 1))
                    nc.vector.tensor_scalar_mul(
                        M1blk[sl, C * b:C * (b + 1)], M1u_ps[sl, 0:C],
                        ksC[sl, :])

                if DEBUG and blk == DBG_BLK:
                    gtmp = smal.tile([P, EXT], F32, tag="gdump")
                    nc.vector.tensor_copy(gtmp[:], G_ps[:])
                    nc.sync.dma_start(dbg_G[:], gtmp[:])
                    nc.sync.dma_start(dbg_ksC[:], ksC[:])
                    nc.sync.dma_start(dbg_M1[:], M1blk[:])
                # ===================== phase 2 (channel-major) ==========
                last = blk == NB - 1
                if not last:
                    he_next = []
                    for s in range(NSL):
                        hx = hex_.tile([P, NCH * EXT], BF16, tag="hex")
                        nc.gpsimd.memset(
                            hx[:].rearrange("p (c l) -> p c l", l=EXT)
                            [:, :, 128:129], 1.0)
                        he_next.append(hx)
                for t in range(NZ):
                    z1 = zps.tile([P, ZW], F32, tag="z")
                    for hw in range(2):
                        cs = ZW * t + 512 * hw
                        nc.tensor.matmul(
                            z1[:, 512 * hw:512 * (hw + 1)], M1blk[:],
                            q_cm[:, cs:cs + 512],
                        )
                    a1 = a1p.tile([P, ZW], BF16, tag="a1")
                    nc.scalar.activation(a1[:], z1[:], AF.Gelu,
                                         bias=b1rep[blk][:])
                    z2 = zps.tile([P, ZW], F32, tag="z")
                    for hw in range(2):
                        nc.tensor.matmul(
                            z2[:, 512 * hw:512 * (hw + 1)], W2blk[blk][:],
                            a1[:, 512 * hw:512 * (hw + 1)],
                        )
                    if t % 2 == 0:
                        hn = hcm.tile([P, SLC], BF16, tag="hcm")
                    ho = hn[:, ZW * (t % 2):ZW * (t % 2 + 1)]
                    if last:
                        nc.scalar.activation(
                            ho, z2[:], AF.Gelu, bias=b2rep[blk][:],
                            accum_out=pooled_parts[:, t:t + 1],
                        )
                    else:
                        nc.scalar.activation(
                            ho, z2[:], AF.Gelu, bias=b2rep[blk][:],
                        )
                    if DEBUG and blk == 0 and t == 0:
                        nc.sync.dma_start(dbg_q[:], q_cm[:, 0:SLC])
                    if DEBUG and blk == 0 and t == 1:
                        nc.sync.dma_start(dbg_h1[:], hn[:])
                    if t % 2 == 1 and not last:
                        # next block's token-major h via PE transposes
                        s = t // 2
                        hx_v = he_next[s][:].rearrange(
                            "p (c l) -> p c l", l=EXT)
                        for g in range(2):
                            hq = qps.tile([P, ZW], BF16, tag="qp")
                            for k in range(8):
                                c = 8 * g + k
                                nc.tensor.transpose(
                                    hq[:, 128 * k:128 * (k + 1)],
                                    hn[:, 128 * c:128 * (c + 1)],
                                    ident[:],
                                )
                            nc.vector.tensor_copy(
                                hx_v[:, 8 * g:8 * (g + 1), 0:128],
                                hq[:].rearrange("p (c l) -> p c l", l=128),
                            )

            # ===================== head =============================
            psum_ = smal.tile([P, 1], F32, tag="poolsum")
            nc.vector.reduce_sum(psum_[:], pooled_parts[:],
                                 axis=mybir.AxisListType.X)
            y_ps = tps.tile([P, C], F32, tag="tiny")
            for b in range(B_LOC):
                sl = slice(C * b, C * (b + 1))
                nc.tensor.matmul(
                    y_ps[sl, 0:1], Whrep[sl, :], psum_[sl, :],
                    tile_position=(C * b, C * b),
                )
            ybn = smal.tile([P, 1], F32, tag="ybn")
            nc.vector.tensor_scalar(
                ybn[:], y_ps[:, 0:1], svecL[:], tvec[:], op0=MUL, op1=ADD,
            )
            yg = smal.tile([P, 1], F32, tag="yg")
            nc.scalar.activation(yg[:], ybn[:], AF.Gelu)
            o_ps = tps.tile([P, C], F32, tag="tiny")
            for b in range(B_LOC):
                nc.tensor.matmul(
                    o_ps[C * b:C * b + DOUT, 0:1],
                    Wfrep[C * b:C * (b + 1), :],
                    yg[C * b:C * (b + 1), :],
                    tile_position=(C * b, C * b),
                )
            ob = smal.tile([P, 1], F32, tag="ob")
            for b in range(B_LOC):
                sl = slice(C * b, C * b + DOUT)
                nc.vector.tensor_tensor(ob[sl, :], o_ps[sl, 0:1], bf_s[sl, :],
                                        op=ADD)
            for b in range(B_LOC):
                nc.sync.dma_start(
                    out_d[b, :], ob[C * b:C * b + DOUT, 0],
                )

    _fix_sync_waits(nc)
    return nc


def _derive_params(arrs):
    """Host-side precompute: replicated weight strips + BN eval folding."""
    tile4 = lambda a: np.tile(a, (B_LOC,) + (1,) * (a.ndim - 1))
    W1 = arrs["fcW1"]            # [NB, C, C]
    W2 = arrs["fcW2"]
    W2b = np.zeros((NB, P, P), np.float32)
    for i in range(NB):
        for b in range(B_LOC):
            W2b[i, C * b:C * (b + 1), C * b:C * (b + 1)] = W2[i]
    svec = arrs["bn_gamma"] / np.sqrt(arrs["bn_var"] + BN_EPS)
    tv = (arrs["bh"] - arrs["bn_mean"]) * svec + arrs["bn_beta"]
    bfr = np.zeros(P, np.float32)
    for b in range(B_LOC):
        bfr[C * b:C * b + DOUT] = arrs["bf"]
    return {
        "W1r": np.ascontiguousarray(
            np.stack([tile4(W1[i]) for i in range(NB)])),
        "W2b": W2b,
        "b1r": np.ascontiguousarray(
            np.stack([tile4(arrs["fcb1"][i]) for i in range(NB)])),
        "b2r": np.ascontiguousarray(
            np.stack([tile4(arrs["fcb2"][i]) for i in range(NB)])),
        "Whr": tile4(arrs["Wh"]),
        "Wfr": tile4(arrs["Wf"]),
        "svecLr": tile4(svec / L),
        "tvecr": tile4(tv),
        "bfr": bfr,
    }


_NC_CACHE = [None]


def kernel(**inputs) -> np.ndarray:
    arrs = {k: np.asarray(v, dtype=np.float32) for k, v in inputs.items()}
    x = arrs["x"]
    B = x.shape[0]
    n_cores = 8
    bl = B // n_cores

    if _NC_CACHE[0] is None:
        _NC_CACHE[0] = build_program()
    nc = _NC_CACHE[0]

    params = _derive_params(arrs)
    in_maps = [
        {"x": np.ascontiguousarray(x[bl * i: bl * (i + 1)]), **params}
        for i in range(n_cores)
    ]
    res = run_bass_kernel_spmd(nc, in_maps, list(range(n_cores))).results
    return np.concatenate([res[i]["out"] for i in range(n_cores)], axis=0)
